# revision 4
# baseline (speedup 1.0000x reference)
"""Causal attention kernel for 8 TRN2 NeuronCores.

Problem: B=4, S=4096, D=1024 single-head causal attention with QKV projection.
  q/k/v = x @ W{q,k,v}.T ; out = softmax(tril(q k^T)/sqrt(D)) @ v

Sharding: core c -> batch b = c//2, parity p = c%2. Each core owns the 16 seq
blocks (128 rows) of batch b with block-index parity p ("striped" sequence
parallelism -> balanced causal work). Q and V are projected fused in a single
pass over the core's own rows; v halves are exchanged between the two cores of
a batch with pair-wise AllGathers issued mid-pass (hidden under the remaining
projection matmuls). No K projection: scores come from s^T = x^T . H with
H = (q Wk)^T built per attention group.

v2 changes over the first working version:
  - Q and V passes fused over one x-chunk stream (x loaded once, not twice);
    the two v AllGathers are issued at 50%/100% of the projection pass so both
    complete long before the PV matmuls need them.
  - Head staging: the first matmul only waits for wq's ec=0 slice (256 KB) and
    per-dc x pieces instead of a 5 MB serialized preload.
  - Causal band trimming: band key blocks only compute score columns q >= j'*128
    (variable-width matmuls) and PV only accumulates the covered q blocks. The
    per-core causal pattern is pushed into data (a [128, 8, 128] sub-block mask:
    triangle on the diagonal, ones/zeros off-diagonal depending on parity) so
    the SPMD program stays identical on all cores.
  - Output written bf16 (host upcasts) halving the tail DMA.
  - Per-(qb,eh) PV eviction as soon as that accumulator's last key block is
    done, overlapping the output DMA with the remaining PV matmuls.
"""

import sys
import types

import numpy as np

sys.path.insert(0, "/opt/trn_rl_repo")

try:
    import antenv.axon_hooks  # noqa: F401
except ImportError:
    _hook_mod = types.ModuleType("antenv.axon_hooks")
    _hook_mod._hook = None
    _hook_mod.set_axon_ntff_profile_hook = (
        lambda h: setattr(_hook_mod, "_hook", h)
    )
    _hook_mod.get_axon_ntff_profile_hook = lambda: _hook_mod._hook
    sys.modules["antenv.axon_hooks"] = _hook_mod

import concourse.bass as bass  # noqa: E402
import concourse.mybir as mybir  # noqa: E402
import concourse.tile as tile  # noqa: E402
from concourse import bacc  # noqa: E402
from concourse.bass_utils import run_bass_kernel_spmd  # noqa: E402
from concourse.masks import make_identity  # noqa: E402

import ml_dtypes  # noqa: E402

B, S, D = 4, 4096, 1024
P = 128
NB = S // P          # 32 seq blocks per batch
NLB = NB // 2        # 16 own blocks per core
SH = S // 2          # 2048 own rows per core
NG = 4               # attention q-groups of 512 rows (4 local blocks each)
SCALE = 1.0 / 32.0   # 1/sqrt(D)

BF16 = mybir.dt.bfloat16
F32 = mybir.dt.float32

_built = {}


def _kb_schedule(g):
    """Key-block schedule for group g: list of (half, o, q0, mr).
    q0: first valid score column (columns < q0 are skipped).
    mr: sub-block mask row (half*4+j') applied to cols [q0, q0+128), or None.
    Ordered q0-ascending so the first block covers every q column."""
    kbs = []
    for o in range(4 * g):            # full blocks, no mask
        for half in (0, 1):
            kbs.append((half, o, 0, None))
    for j in range(4):                # band blocks
        for half in (0, 1):
            kbs.append((half, 4 * g + j, j * P, half * 4 + j))
    return kbs


def _build_nc():
    nc = bacc.Bacc("TRN2", target_bir_lowering=False, debug=False, num_devices=8)

    # All large inputs are laid out partition-major by the host so that each
    # DMA is 128 contiguous per-partition descriptors.
    xtf = nc.declare_dram_parameter("xtf", [8, P, 8 * 512], BF16, isOutput=False)
    xto = nc.declare_dram_parameter("xto", [4, P, 8 * 512], BF16, isOutput=False)
    wqt = nc.declare_dram_parameter("wqt", [P, 8, 8, P], BF16, isOutput=False)
    wkt = nc.declare_dram_parameter("wkt", [P, 8, D], BF16, isOutput=False)
    wvt = nc.declare_dram_parameter("wvt", [P, 8, D], BF16, isOutput=False)
    bmask = nc.declare_dram_parameter("bmask", [P, 8, P], BF16, isOutput=False)
    y = nc.declare_dram_parameter("y", [SH, D], BF16, isOutput=True)

    xtf3 = xtf.ap().rearrange("c p (po s) -> c p po s", po=8)   # [8, 128, 8, 512]
    xto3 = xto.ap().rearrange("c p (po s) -> c p po s", po=8)   # [4, 128, 8, 512]
    wqt3 = wqt.ap()
    wkt3 = wkt.ap()
    wvt3 = wvt.ap()
    bmask3 = bmask.ap()
    y3 = y.ap().rearrange("(nb pi) e -> nb pi e", pi=P)         # [16, 128, 1024]

    PAIRS = [[0, 1], [2, 3], [4, 5], [6, 7]]

    with tile.TileContext(nc) as tc:
        with (
            tc.tile_pool(name="dram", bufs=1, space="DRAM") as dram,
            tc.tile_pool(name="consts", bufs=1) as consts,
            tc.tile_pool(name="wqp", bufs=1) as wqp,
            tc.tile_pool(name="wkp", bufs=1) as wkp,
            tc.tile_pool(name="wvp", bufs=1) as wvp,
            tc.tile_pool(name="hp", bufs=1) as hp,
            tc.tile_pool(name="xtp", bufs=2) as xtp,
            tc.tile_pool(name="qgp", bufs=2) as qgp,
            tc.tile_pool(name="ktp", bufs=1) as ktp,
            tc.tile_pool(name="stg", bufs=3) as stg,
            tc.tile_pool(name="strip", bufs=32) as strip,
            tc.tile_pool(name="vload", bufs=4) as vload,
            tc.tile_pool(name="linvp", bufs=2) as linvp,
            tc.tile_pool(name="ctxs", bufs=3) as ctxs,
            tc.tile_pool(name="psum", bufs=8, space="PSUM") as psum,
        ):
            v_own = dram.tile([NLB, P, D], BF16, tag="v_own", name="v_own")
            v_all_a = dram.tile([NLB, P, D], BF16, tag="v_all_a", name="v_all_a")
            v_all_b = dram.tile([NLB, P, D], BF16, tag="v_all_b", name="v_all_b")
            qt_dram = dram.tile([NG, P, 8, 512], BF16, tag="qt_dram", name="qt_dram")

            tri_sb = consts.tile([P, 8, P], BF16)
            ones_sb = consts.tile([P, P], BF16)
            nc.gpsimd.memset(ones_sb[:], 1.0)
            ident_sb = consts.tile([P, P], F32)
            make_identity(nc, ident_sb[:])

            xt_sb = ktp.tile([P, 8, S], BF16)        # x^T: [d, all 4096 rows]

            # ---- Head staging: the very first PSUM group (ec=0) only needs
            # wq's ec=0 slice and the 8 per-dc pieces of the first x chunk, so
            # those go first on the sync queue in chase-able pieces.
            wq_sb = wqp.tile([P, 8, 8, P], BF16, name="wq_sb")
            nc.sync.dma_start(wq_sb[:, 0], wqt3[:, 0])
            xt_first = xtp.tile([P, 8, 512], BF16, tag="xt", name="xt_first")
            for dc in range(8):
                nc.sync.dma_start(xt_first[:, dc], xto3[0, :, dc])
            nc.sync.dma_start(wq_sb[:, 1:8], wqt3[:, 1:8])
            wk_sb = wkp.tile([P, 8, D], BF16, name="wk_sb")
            nc.sync.dma_start(wk_sb[:], wkt3)
            wv_sb = wvp.tile([P, 8, D], BF16, name="wv_sb")
            nc.sync.dma_start(wv_sb[:], wvt3)

            # band masks + full-batch x^T stream in the background on queues
            # the projection loads never touch (gpsimd ring drains these well
            # before the gather triggers and vt loads queued behind them)
            nc.scalar.dma_start(tri_sb[:], bmask3)
            for c in (0, 4, 1, 5, 2, 6, 3, 7):
                nc.gpsimd.dma_start(xt_sb[:, :, c * 512:(c + 1) * 512], xtf3[c])

            # ---- Fused Q+V projection pass over the core's own rows.
            qg_tiles = {}

            def load_qg(g):
                qg = qgp.tile([P, 8, 512], BF16, tag="qg", name=f"qg_{g}")
                nc.scalar.dma_start(qg[:], qt_dram[g])
                qg_tiles[g] = qg

            xt_t = xt_first
            for c in range(4):
                if c < 3:  # prefetch next chunk on the scalar queue
                    xt_n = xtp.tile([P, 8, 512], BF16, tag="xt", name="xt_t")
                    nc.scalar.dma_start(xt_n[:, 0:4], xto3[c + 1, :, 0:4])
                    nc.scalar.dma_start(xt_n[:, 4:8], xto3[c + 1, :, 4:8])
                # Q: out q^T[e, s] for this 512-row chunk
                for ec in range(8):
                    ps = psum.tile([P, 512], F32, tag="bank", name="ps_q")
                    for dc in range(8):
                        nc.tensor.matmul(
                            ps[:],
                            lhsT=wq_sb[:, ec, dc],
                            rhs=xt_t[:, dc, :],
                            start=(dc == 0),
                            stop=(dc == 7),
                        )
                    qs = stg.tile([P, 512], BF16, tag="stg512", name="qs")
                    nc.scalar.mul(qs[:], ps[:], 1.0)
                    nc.sync.dma_start(qt_dram[c, :, ec, :], qs[:])
                # V: out v[s, e] for the same chunk
                for eh in range(2):
                    for sb in range(4):
                        ps = psum.tile([P, 512], F32, tag="bank", name="ps_v")
                        for dc in range(8):
                            nc.tensor.matmul(
                                ps[:],
                                lhsT=xt_t[:, dc, sb * P:(sb + 1) * P],
                                rhs=wv_sb[:, dc, eh * 512:(eh + 1) * 512],
                                start=(dc == 0),
                                stop=(dc == 7),
                            )
                        vho = stg.tile([P, 512], BF16, tag="stg512", name="vho")
                        nc.vector.tensor_copy(out=vho[:], in_=ps[:])
                        nc.sync.dma_start(
                            v_own[c * 4 + sb][:, eh * 512:(eh + 1) * 512], vho[:]
                        )
                if c == 0:
                    load_qg(0)
                if c == 1:
                    # first half-gather (own blocks 0-7) issued mid-pass so it
                    # completes well before PV group 0 reads it
                    nc.gpsimd.collective_compute(
                        "AllGather",
                        mybir.AluOpType.bypass,
                        replica_groups=PAIRS,
                        ins=[v_own[0:8].opt()],
                        outs=[v_all_a[:].opt()],
                    )
                if c == 2:
                    load_qg(1)
                if c == 3:
                    nc.gpsimd.collective_compute(
                        "AllGather",
                        mybir.AluOpType.bypass,
                        replica_groups=PAIRS,
                        ins=[v_own[8:16].opt()],
                        outs=[v_all_b[:].opt()],
                    )
                xt_t = xt_n

            # ---- Attention ----
            def pass1(g):
                """QK + exp + mask + denominator for group g."""
                kbs = _kb_schedule(g)
                nkb = len(kbs)
                qg = qg_tiles[g]

                # H[d, qi] = sum_e Wk[e, d] q[qi, e], evicted bf16 to SBUF
                h_sb = hp.tile([P, 8, 512], BF16, tag="h", name=f"h_{g}")
                for db in range(8):
                    hps = psum.tile([P, 512], F32, tag="bank", name=f"hps_{g}_{db}")
                    for ec in range(8):
                        nc.tensor.matmul(
                            hps[:],
                            lhsT=wk_sb[:, ec, db * P:(db + 1) * P],
                            rhs=qg[:, ec, :],
                            start=(ec == 0),
                            stop=(ec == 7),
                        )
                    nc.vector.tensor_copy(out=h_sb[:, db, :], in_=hps[:])

                if g + 2 < NG:
                    load_qg(g + 2)

                lrep_ps = psum.tile([P, 512], F32, tag="bank", name=f"lrep_{g}")
                pts = []

                def l_accum(kb_idx):
                    # denominator: column sums replicated across partitions,
                    # issued one key block late so the PE never waits on the
                    # exp/mask of the block it just produced. Column ranges
                    # vary per block; kb 0 is full-width with start=True so
                    # every column's has_written bit is set before any
                    # narrower accumulation lands on it.
                    q0 = kbs[kb_idx][2]
                    nc.tensor.matmul(
                        lrep_ps[:, q0:],
                        lhsT=ones_sb[:],
                        rhs=pts[kb_idx][:, q0:],
                        start=(kb_idx == 0),
                        stop=(kb_idx == nkb - 1),
                        skip_group_check=True,
                    )

                for kb_idx, (half, o, q0, mr) in enumerate(kbs):
                    kcol = half * SH + o * P
                    st_ps = psum.tile([P, 512], F32, tag="bank", name=f"st_ps_{g}")
                    for dc in range(8):
                        nc.tensor.matmul(
                            st_ps[:, q0:],
                            lhsT=xt_sb[:, dc, kcol:kcol + P],
                            rhs=h_sb[:, dc, q0:],
                            start=(dc == 0),
                            stop=(dc == 7),
                        )
                    pt = strip.tile([P, 512], BF16, tag="pt", name=f"pt_{g}")
                    nc.scalar.activation(
                        pt[:, q0:], st_ps[:, q0:],
                        mybir.ActivationFunctionType.Exp, scale=SCALE
                    )
                    if mr is not None:  # mask cols [q0, q0+128): tri/ones/zeros
                        nc.vector.tensor_mul(
                            out=pt[:, q0:q0 + P], in0=pt[:, q0:q0 + P],
                            in1=tri_sb[:, mr, :],
                        )
                    pts.append(pt)
                    if kb_idx >= 1:
                        l_accum(kb_idx - 1)
                l_accum(nkb - 1)

                # denominator -> per-partition scalars via PE transpose of the
                # row-replicated sums; 1/l folds into the ctx eviction scale.
                lsb = linvp.tile([P, 512], F32, tag="lsb", bufs=1, name=f"lsb_{g}")
                nc.vector.tensor_copy(out=lsb[:], in_=lrep_ps[:])
                linv_col = []
                for qb in range(4):
                    ltr = psum.tile([P, P], F32, tag="bank", name=f"ltr_{g}_{qb}")
                    nc.tensor.transpose(ltr[:], lsb[:, qb * P:(qb + 1) * P], ident_sb[:])
                    lc = linvp.tile([P, 1], F32, tag="linv", bufs=8, name=f"linv_{g}_{qb}")
                    nc.vector.reciprocal(lc[:], ltr[:, 0:1])
                    linv_col.append(lc)
                return kbs, pts, linv_col

            def pv(g, state):
                kbs, pts, linv_col = state
                # last key block covering each q sub-block (kb covers qb iff
                # its q0 <= qb*128); evict that accumulator right after.
                last_cover = [max(i for i, kb in enumerate(kbs) if kb[2] <= qb * P)
                              for qb in range(4)]
                ctx_ps = {
                    (qb, eh): psum.tile([P, 512], F32, tag="bank",
                                        name=f"ctx_{g}_{qb}_{eh}")
                    for qb in range(4) for eh in range(2)
                }
                ectr = 0
                for kb_idx, (half, o, q0, mr) in enumerate(kbs):
                    vsrc = v_all_a if o < 8 else v_all_b
                    vb = half * 8 + (o % 8)
                    vt = vload.tile([P, D], BF16, tag="vt", name=f"vt_{g}")
                    # gpsimd: these DMAs wait on the v AllGather semaphore; on
                    # the in-order sync DMA stream they would head-of-line
                    # block later DMAs.
                    nc.gpsimd.dma_start(vt[:], vsrc[vb])
                    for qb in range(4):
                        if q0 > qb * P:
                            continue  # this key block covers no rows of qb
                        for eh in range(2):
                            nc.tensor.matmul(
                                ctx_ps[(qb, eh)][:],
                                lhsT=pts[kb_idx][:, qb * P:(qb + 1) * P],
                                rhs=vt[:, eh * 512:(eh + 1) * 512],
                                start=(kb_idx == 0),
                                stop=(kb_idx == last_cover[qb]),
                            )
                    for qb in range(4):
                        if last_cover[qb] != kb_idx:
                            continue
                        for eh in range(2):
                            cs = ctxs.tile([P, 512], BF16, tag="cs", name=f"cs_{g}")
                            # normalize during eviction; alternate engines so
                            # PSUM banks free ~2x faster
                            if ectr % 2 == 0:
                                nc.scalar.mul(cs[:], ctx_ps[(qb, eh)][:], linv_col[qb][:])
                            else:
                                nc.vector.tensor_scalar_mul(
                                    cs[:], ctx_ps[(qb, eh)][:], linv_col[qb][:])
                            ectr += 1
                            nc.sync.dma_start(
                                y3[4 * g + qb, :, eh * 512:(eh + 1) * 512], cs[:]
                            )

            for g in range(NG):
                pv(g, pass1(g))

    nc.compile()
    return nc


def _host_inputs(x, Wq, Wk, Wv):
    """Build per-core input maps. x: [B,S,D] f32; W*: [D,D] f32."""
    bf = ml_dtypes.bfloat16
    # wq: [pi, ec, dc, e''] with element = Wq[ec*128+e'', dc*128+pi]
    wqt = np.ascontiguousarray(
        Wq.T.astype(bf).reshape(8, P, 8, P).transpose(1, 2, 0, 3)
    )
    # wk: [pi, ec, d] with element = Wk[ec*128+pi, d]
    wkt = np.ascontiguousarray(Wk.astype(bf).reshape(8, P, D).transpose(1, 0, 2))
    # wv: [pi, dc, e] with element = Wv^T[dc*128+pi, e] = Wv[e, dc*128+pi]
    wvt = np.ascontiguousarray(Wv.T.astype(bf).reshape(8, P, D).transpose(1, 0, 2))

    kj = np.arange(P)[:, None]
    qrow = np.arange(P)[None, :]
    tri = (kj <= qrow).astype(np.float32)

    in_maps = []
    xb_cache = {}
    for c in range(8):
        b, p = c // 2, c % 2
        if b not in xb_cache:
            # parity order: [even blocks | odd blocks]
            perm = [2 * j for j in range(NLB)] + [2 * j + 1 for j in range(NLB)]
            xbf = x[b].reshape(NB, P, D)[perm].reshape(S, D)
            xb_cache[b] = xbf.T.astype(bf)  # [D, S]
        xt_full = xb_cache[b]
        # [c, pi, po*512]: per-partition-contiguous chunks
        xtf_c = np.ascontiguousarray(
            xt_full.reshape(8, P, 8, 512).transpose(2, 1, 0, 3)
        ).reshape(8, P, 8 * 512)
        xto_half = xt_full[:, p * SH:(p + 1) * SH]
        xto_c = np.ascontiguousarray(
            xto_half.reshape(8, P, 4, 512).transpose(2, 1, 0, 3)
        ).reshape(4, P, 8 * 512)

        # band sub-block masks [128 kj, 8 r=half*4+j', 128 qrow]:
        # half==p: diagonal triangle; half<p: keep all; half>p: drop all.
        bmask = np.zeros((P, 8, P), np.float32)
        for half in (0, 1):
            for j in range(4):
                if half == p:
                    bmask[:, half * 4 + j, :] = tri
                elif half < p:
                    bmask[:, half * 4 + j, :] = 1.0
        in_maps.append({
            "xtf": xtf_c,
            "xto": xto_c,
            "wqt": wqt,
            "wkt": wkt,
            "wvt": wvt,
            "bmask": bmask.astype(bf),
        })
    return in_maps


def kernel(**inputs):
    x = np.asarray(inputs["inputs"], np.float32)
    Wq = np.asarray(inputs["Wq"], np.float32)
    Wk = np.asarray(inputs["Wk"], np.float32)
    Wv = np.asarray(inputs["Wv"], np.float32)

    if "nc" not in _built:
        _built["nc"] = _build_nc()
    nc = _built["nc"]

    in_maps = _host_inputs(x, Wq, Wk, Wv)
    res = run_bass_kernel_spmd(nc, in_maps, core_ids=list(range(8)))

    out = np.empty((B, S, D), np.float32)
    for c in range(8):
        b, p = c // 2, c % 2
        yc = res.results[c]["y"].astype(np.float32).reshape(NLB, P, D)
        ob = out[b].reshape(NB, P, D)
        for j in range(NLB):
            ob[2 * j + p] = yc[j]
    return out


# revision 7
# speedup vs baseline: 1.1599x; 1.1599x over previous
"""Causal attention kernel for 8 TRN2 NeuronCores.

Problem: B=4, S=4096, D=1024 single-head causal attention with QKV projection.
  q/k/v = x @ W{q,k,v}.T ; out = softmax(tril(q k^T)/sqrt(D)) @ v

Sharding: core c -> batch b = c//2, parity p = c%2. Each core owns the 16 seq
blocks (128 rows) of batch b with block-index parity p ("striped" sequence
parallelism -> balanced causal work). Q and V are projected fused in a single
pass over the core's own rows; v halves are exchanged between the two cores of
a batch with pair-wise AllGathers issued mid-pass (hidden under the remaining
projection matmuls). No K projection: scores come from s^T = x^T . H with
H = (q Wk)^T built per attention group.

v2 changes over the first working version:
  - Q and V passes fused over one x-chunk stream (x loaded once, not twice);
    the two v AllGathers are issued at 50%/100% of the projection pass so both
    complete long before the PV matmuls need them.
  - Head staging: the first matmul only waits for wq's ec=0 slice (256 KB) and
    per-dc x pieces instead of a 5 MB serialized preload.
  - Causal band trimming: band key blocks only compute score columns q >= j'*128
    (variable-width matmuls) and PV only accumulates the covered q blocks. The
    per-core causal pattern is pushed into data (a [128, 8, 128] sub-block mask:
    triangle on the diagonal, ones/zeros off-diagonal depending on parity) so
    the SPMD program stays identical on all cores.
  - Output written bf16 (host upcasts) halving the tail DMA.
  - Per-(qb,eh) PV eviction as soon as that accumulator's last key block is
    done, overlapping the output DMA with the remaining PV matmuls.
"""

import sys
import types

import numpy as np

sys.path.insert(0, "/opt/trn_rl_repo")

try:
    import antenv.axon_hooks  # noqa: F401
except ImportError:
    _hook_mod = types.ModuleType("antenv.axon_hooks")
    _hook_mod._hook = None
    _hook_mod.set_axon_ntff_profile_hook = (
        lambda h: setattr(_hook_mod, "_hook", h)
    )
    _hook_mod.get_axon_ntff_profile_hook = lambda: _hook_mod._hook
    sys.modules["antenv.axon_hooks"] = _hook_mod

import concourse.bass as bass  # noqa: E402
import concourse.mybir as mybir  # noqa: E402
import concourse.tile as tile  # noqa: E402
from concourse import bacc  # noqa: E402
from concourse.bass_utils import run_bass_kernel_spmd  # noqa: E402
from concourse.masks import make_identity  # noqa: E402

import ml_dtypes  # noqa: E402

B, S, D = 4, 4096, 1024
P = 128
NB = S // P          # 32 seq blocks per batch
NLB = NB // 2        # 16 own blocks per core
SH = S // 2          # 2048 own rows per core
NG = 4               # attention q-groups of 512 rows (4 local blocks each)
SCALE = 1.0 / 32.0   # 1/sqrt(D)

BF16 = mybir.dt.bfloat16
F32 = mybir.dt.float32

_built = {}


def _kb_schedule(g):
    """Key-block schedule for group g: list of (half, o, q0, mr).
    q0: first valid score column (columns < q0 are skipped).
    mr: sub-block mask row (half*4+j') applied to cols [q0, q0+128), or None.
    Ordered q0-ascending so the first block covers every q column."""
    kbs = []
    for o in range(4 * g):            # full blocks, no mask
        for half in (0, 1):
            kbs.append((half, o, 0, None))
    for j in range(4):                # band blocks
        for half in (0, 1):
            kbs.append((half, 4 * g + j, j * P, half * 4 + j))
    return kbs


def _build_nc():
    nc = bacc.Bacc("TRN2", target_bir_lowering=False, debug=False, num_devices=8)

    # All large inputs are laid out partition-major by the host so that each
    # DMA is 128 contiguous per-partition descriptors.
    xtf = nc.declare_dram_parameter("xtf", [8, P, 8 * 512], BF16, isOutput=False)
    xto = nc.declare_dram_parameter("xto", [4, P, 8 * 512], BF16, isOutput=False)
    wqt = nc.declare_dram_parameter("wqt", [P, 8, 8, P], BF16, isOutput=False)
    wkt = nc.declare_dram_parameter("wkt", [P, 8, D], BF16, isOutput=False)
    wvt = nc.declare_dram_parameter("wvt", [P, 8, D], BF16, isOutput=False)
    bmask = nc.declare_dram_parameter("bmask", [P, 8, P], BF16, isOutput=False)
    y = nc.declare_dram_parameter("y", [SH, D], BF16, isOutput=True)

    xtf3 = xtf.ap().rearrange("c p (po s) -> c p po s", po=8)   # [8, 128, 8, 512]
    xto3 = xto.ap().rearrange("c p (po s) -> c p po s", po=8)   # [4, 128, 8, 512]
    wqt3 = wqt.ap()
    wkt3 = wkt.ap()
    wvt3 = wvt.ap()
    bmask3 = bmask.ap()
    y3 = y.ap().rearrange("(nb pi) e -> nb pi e", pi=P)         # [16, 128, 1024]

    PAIRS = [[0, 1], [2, 3], [4, 5], [6, 7]]

    with tile.TileContext(nc) as tc:
        with (
            tc.tile_pool(name="dram", bufs=1, space="DRAM") as dram,
            tc.tile_pool(name="consts", bufs=1) as consts,
            tc.tile_pool(name="wqp", bufs=1) as wqp,
            tc.tile_pool(name="wkp", bufs=1) as wkp,
            tc.tile_pool(name="wvp", bufs=1) as wvp,
            tc.tile_pool(name="hp", bufs=1) as hp,
            tc.tile_pool(name="xtp", bufs=2) as xtp,
            tc.tile_pool(name="qgp", bufs=2) as qgp,
            tc.tile_pool(name="ktp", bufs=1) as ktp,
            tc.tile_pool(name="stg", bufs=3) as stg,
            tc.tile_pool(name="strip", bufs=32) as strip,
            tc.tile_pool(name="vload", bufs=4) as vload,
            tc.tile_pool(name="linvp", bufs=2) as linvp,
            tc.tile_pool(name="ctxs", bufs=3) as ctxs,
            tc.tile_pool(name="psum", bufs=8, space="PSUM") as psum,
        ):
            v_own = dram.tile([NLB, P, D], BF16, tag="v_own", name="v_own")
            v_all_a = dram.tile([NLB, P, D], BF16, tag="v_all_a", name="v_all_a")
            v_all_b = dram.tile([NLB, P, D], BF16, tag="v_all_b", name="v_all_b")
            qt_dram = dram.tile([NG, P, 8, 512], BF16, tag="qt_dram", name="qt_dram")

            tri_sb = consts.tile([P, 8, P], BF16)
            ones_sb = consts.tile([P, P], BF16)
            nc.gpsimd.memset(ones_sb[:], 1.0)
            ident_sb = consts.tile([P, P], F32)
            make_identity(nc, ident_sb[:])

            xt_sb = ktp.tile([P, 8, S], BF16)        # x^T: [d, all 4096 rows]

            # ---- Head staging. Each DMA ring entry costs ~0.5 us of ring
            # latency on top of its transfer time, so the head is cut into
            # just enough pieces that the PE's chunk-0 chase (ec groups every
            # 1.7 us, then the V dc-chase) is never more than one piece ahead
            # of the ring: wq ec=0 first, x chunk 0 in 4 pieces, the other wq
            # ec slices individually, then wv in eh halves. wk is NOT in the
            # head -- it is first read ~120 us in by the H matmuls and is
            # inserted into the ring between the chunk-1 and chunk-2 eviction
            # writes instead.
            wq_sb = wqp.tile([P, 8, 8, P], BF16, name="wq_sb")
            nc.sync.dma_start(wq_sb[:, 0], wqt3[:, 0])
            xt_first = xtp.tile([P, 8, 512], BF16, tag="xt", name="xt_first")
            for dc in range(0, 8, 2):
                nc.sync.dma_start(xt_first[:, dc:dc + 2], xto3[0, :, dc:dc + 2])
            for ec in range(1, 8):
                nc.sync.dma_start(wq_sb[:, ec], wqt3[:, ec])
            wv_sb = wvp.tile([P, 8, D], BF16, name="wv_sb")
            nc.sync.dma_start(wv_sb[:, :, 0:512], wvt3[:, :, 0:512])
            nc.sync.dma_start(wv_sb[:, :, 512:D], wvt3[:, :, 512:D])
            wk_sb = wkp.tile([P, 8, D], BF16, name="wk_sb")

            # band masks on the scalar ring; most of the full-batch x^T on the
            # (slow, ~64 GB/s) gpsimd ring -- it has ~95 us before the first
            # score matmuls need the later pieces, and the gather triggers
            # queued behind it still fire in time. The two pieces group 0
            # reads first go on the fast sync ring mid-pass instead.
            nc.scalar.dma_start(tri_sb[:], bmask3)
            for c in (1, 5, 2, 6, 3, 7):
                nc.gpsimd.dma_start(xt_sb[:, :, c * 512:(c + 1) * 512], xtf3[c])

            # ---- Fused Q+V projection pass over the core's own rows.
            qg_tiles = {}

            def load_qg(g):
                qg = qgp.tile([P, 8, 512], BF16, tag="qg", name=f"qg_{g}")
                nc.scalar.dma_start(qg[:], qt_dram[g])
                qg_tiles[g] = qg

            xt_t = xt_first
            for c in range(4):
                if c < 3:  # prefetch next chunk on the scalar queue
                    xt_n = xtp.tile([P, 8, 512], BF16, tag="xt", name="xt_t")
                    nc.scalar.dma_start(xt_n[:, 0:4], xto3[c + 1, :, 0:4])
                    nc.scalar.dma_start(xt_n[:, 4:8], xto3[c + 1, :, 4:8])
                # Q: out q^T[e, s] for this 512-row chunk
                for ec in range(8):
                    ps = psum.tile([P, 512], F32, tag="bank", name="ps_q")
                    for dc in range(8):
                        nc.tensor.matmul(
                            ps[:],
                            lhsT=wq_sb[:, ec, dc],
                            rhs=xt_t[:, dc, :],
                            start=(dc == 0),
                            stop=(dc == 7),
                        )
                    qs = stg.tile([P, 512], BF16, tag="stg512", name="qs")
                    nc.scalar.mul(qs[:], ps[:], 1.0)
                    nc.sync.dma_start(qt_dram[c, :, ec, :], qs[:])
                # V: out v[s, e] for the same chunk
                for eh in range(2):
                    for sb in range(4):
                        ps = psum.tile([P, 512], F32, tag="bank", name="ps_v")
                        for dc in range(8):
                            nc.tensor.matmul(
                                ps[:],
                                lhsT=xt_t[:, dc, sb * P:(sb + 1) * P],
                                rhs=wv_sb[:, dc, eh * 512:(eh + 1) * 512],
                                start=(dc == 0),
                                stop=(dc == 7),
                            )
                        vho = stg.tile([P, 512], BF16, tag="stg512", name="vho")
                        nc.vector.tensor_copy(out=vho[:], in_=ps[:])
                        nc.sync.dma_start(
                            v_own[c * 4 + sb][:, eh * 512:(eh + 1) * 512], vho[:]
                        )
                if c == 0:
                    load_qg(0)
                    # group 0's two x^T pieces, on the fast ring behind the
                    # chunk-0 eviction writes
                    for cc in (0, 4):
                        nc.sync.dma_start(
                            xt_sb[:, :, cc * 512:(cc + 1) * 512], xtf3[cc])
                if c == 1:
                    # first half-gather (own blocks 0-7) issued mid-pass so it
                    # completes well before PV group 0 reads it
                    nc.gpsimd.collective_compute(
                        "AllGather",
                        mybir.AluOpType.bypass,
                        replica_groups=PAIRS,
                        ins=[v_own[0:8].opt()],
                        outs=[v_all_a[:].opt()],
                    )
                if c == 1:
                    nc.sync.dma_start(wk_sb[:], wkt3)
                if c == 2:
                    load_qg(1)
                if c == 3:
                    nc.gpsimd.collective_compute(
                        "AllGather",
                        mybir.AluOpType.bypass,
                        replica_groups=PAIRS,
                        ins=[v_own[8:16].opt()],
                        outs=[v_all_b[:].opt()],
                    )
                xt_t = xt_n

            # ---- Attention ----
            def pass1(g):
                """QK + exp + mask + denominator for group g."""
                kbs = _kb_schedule(g)
                nkb = len(kbs)
                qg = qg_tiles[g]

                # H[d, qi] = sum_e Wk[e, d] q[qi, e], evicted bf16 to SBUF
                h_sb = hp.tile([P, 8, 512], BF16, tag="h", name=f"h_{g}")
                for db in range(8):
                    hps = psum.tile([P, 512], F32, tag="bank", name=f"hps_{g}_{db}")
                    for ec in range(8):
                        nc.tensor.matmul(
                            hps[:],
                            lhsT=wk_sb[:, ec, db * P:(db + 1) * P],
                            rhs=qg[:, ec, :],
                            start=(ec == 0),
                            stop=(ec == 7),
                        )
                    nc.vector.tensor_copy(out=h_sb[:, db, :], in_=hps[:])

                if g + 2 < NG:
                    load_qg(g + 2)

                lrep_ps = psum.tile([P, 512], F32, tag="bank", name=f"lrep_{g}")
                pts = []

                def l_accum(kb_idx):
                    # denominator: column sums replicated across partitions,
                    # issued one key block late so the PE never waits on the
                    # exp/mask of the block it just produced. Column ranges
                    # vary per block; kb 0 is full-width with start=True so
                    # every column's has_written bit is set before any
                    # narrower accumulation lands on it.
                    q0 = kbs[kb_idx][2]
                    nc.tensor.matmul(
                        lrep_ps[:, q0:],
                        lhsT=ones_sb[:],
                        rhs=pts[kb_idx][:, q0:],
                        start=(kb_idx == 0),
                        stop=(kb_idx == nkb - 1),
                        skip_group_check=True,
                    )

                for kb_idx, (half, o, q0, mr) in enumerate(kbs):
                    kcol = half * SH + o * P
                    st_ps = psum.tile([P, 512], F32, tag="bank", name=f"st_ps_{g}")
                    for dc in range(8):
                        nc.tensor.matmul(
                            st_ps[:, q0:],
                            lhsT=xt_sb[:, dc, kcol:kcol + P],
                            rhs=h_sb[:, dc, q0:],
                            start=(dc == 0),
                            stop=(dc == 7),
                        )
                    pt = strip.tile([P, 512], BF16, tag="pt", name=f"pt_{g}")
                    nc.scalar.activation(
                        pt[:, q0:], st_ps[:, q0:],
                        mybir.ActivationFunctionType.Exp, scale=SCALE
                    )
                    if mr is not None:  # mask cols [q0, q0+128): tri/ones/zeros
                        nc.vector.tensor_mul(
                            out=pt[:, q0:q0 + P], in0=pt[:, q0:q0 + P],
                            in1=tri_sb[:, mr, :],
                        )
                    pts.append(pt)
                    if kb_idx >= 1:
                        l_accum(kb_idx - 1)
                l_accum(nkb - 1)

                # denominator -> per-partition scalars via PE transpose of the
                # row-replicated sums; 1/l folds into the ctx eviction scale.
                lsb = linvp.tile([P, 512], F32, tag="lsb", bufs=1, name=f"lsb_{g}")
                nc.vector.tensor_copy(out=lsb[:], in_=lrep_ps[:])
                linv_col = []
                for qb in range(4):
                    ltr = psum.tile([P, P], F32, tag="bank", name=f"ltr_{g}_{qb}")
                    nc.tensor.transpose(ltr[:], lsb[:, qb * P:(qb + 1) * P], ident_sb[:])
                    lc = linvp.tile([P, 1], F32, tag="linv", bufs=8, name=f"linv_{g}_{qb}")
                    nc.vector.reciprocal(lc[:], ltr[:, 0:1])
                    linv_col.append(lc)
                return kbs, pts, linv_col

            def pv(g, state):
                kbs, pts, linv_col = state
                # last key block covering each q sub-block (kb covers qb iff
                # its q0 <= qb*128); evict that accumulator right after.
                last_cover = [max(i for i, kb in enumerate(kbs) if kb[2] <= qb * P)
                              for qb in range(4)]
                ctx_ps = {
                    (qb, eh): psum.tile([P, 512], F32, tag="bank",
                                        name=f"ctx_{g}_{qb}_{eh}")
                    for qb in range(4) for eh in range(2)
                }
                ectr = 0
                for kb_idx, (half, o, q0, mr) in enumerate(kbs):
                    vsrc = v_all_a if o < 8 else v_all_b
                    vb = half * 8 + (o % 8)
                    vt = vload.tile([P, D], BF16, tag="vt", name=f"vt_{g}")
                    # gpsimd: these DMAs wait on the v AllGather semaphore; on
                    # the in-order sync DMA stream they would head-of-line
                    # block later DMAs.
                    nc.gpsimd.dma_start(vt[:], vsrc[vb])
                    for qb in range(4):
                        if q0 > qb * P:
                            continue  # this key block covers no rows of qb
                        for eh in range(2):
                            nc.tensor.matmul(
                                ctx_ps[(qb, eh)][:],
                                lhsT=pts[kb_idx][:, qb * P:(qb + 1) * P],
                                rhs=vt[:, eh * 512:(eh + 1) * 512],
                                start=(kb_idx == 0),
                                stop=(kb_idx == last_cover[qb]),
                            )
                    for qb in range(4):
                        if last_cover[qb] != kb_idx:
                            continue
                        for eh in range(2):
                            cs = ctxs.tile([P, 512], BF16, tag="cs", name=f"cs_{g}")
                            # normalize during eviction; alternate engines so
                            # PSUM banks free ~2x faster
                            if ectr % 2 == 0:
                                nc.scalar.mul(cs[:], ctx_ps[(qb, eh)][:], linv_col[qb][:])
                            else:
                                nc.vector.tensor_scalar_mul(
                                    cs[:], ctx_ps[(qb, eh)][:], linv_col[qb][:])
                            ectr += 1
                            nc.sync.dma_start(
                                y3[4 * g + qb, :, eh * 512:(eh + 1) * 512], cs[:]
                            )

            for g in range(NG):
                pv(g, pass1(g))

    nc.compile()
    return nc


def _host_inputs(x, Wq, Wk, Wv):
    """Build per-core input maps. x: [B,S,D] f32; W*: [D,D] f32."""
    bf = ml_dtypes.bfloat16
    # wq: [pi, ec, dc, e''] with element = Wq[ec*128+e'', dc*128+pi]
    wqt = np.ascontiguousarray(
        Wq.T.astype(bf).reshape(8, P, 8, P).transpose(1, 2, 0, 3)
    )
    # wk: [pi, ec, d] with element = Wk[ec*128+pi, d]
    wkt = np.ascontiguousarray(Wk.astype(bf).reshape(8, P, D).transpose(1, 0, 2))
    # wv: [pi, dc, e] with element = Wv^T[dc*128+pi, e] = Wv[e, dc*128+pi]
    wvt = np.ascontiguousarray(Wv.T.astype(bf).reshape(8, P, D).transpose(1, 0, 2))

    kj = np.arange(P)[:, None]
    qrow = np.arange(P)[None, :]
    tri = (kj <= qrow).astype(np.float32)

    in_maps = []
    xb_cache = {}
    for c in range(8):
        b, p = c // 2, c % 2
        if b not in xb_cache:
            # parity order: [even blocks | odd blocks]
            perm = [2 * j for j in range(NLB)] + [2 * j + 1 for j in range(NLB)]
            xbf = x[b].reshape(NB, P, D)[perm].reshape(S, D)
            xb_cache[b] = xbf.T.astype(bf)  # [D, S]
        xt_full = xb_cache[b]
        # [c, pi, po*512]: per-partition-contiguous chunks
        xtf_c = np.ascontiguousarray(
            xt_full.reshape(8, P, 8, 512).transpose(2, 1, 0, 3)
        ).reshape(8, P, 8 * 512)
        xto_half = xt_full[:, p * SH:(p + 1) * SH]
        xto_c = np.ascontiguousarray(
            xto_half.reshape(8, P, 4, 512).transpose(2, 1, 0, 3)
        ).reshape(4, P, 8 * 512)

        # band sub-block masks [128 kj, 8 r=half*4+j', 128 qrow]:
        # half==p: diagonal triangle; half<p: keep all; half>p: drop all.
        bmask = np.zeros((P, 8, P), np.float32)
        for half in (0, 1):
            for j in range(4):
                if half == p:
                    bmask[:, half * 4 + j, :] = tri
                elif half < p:
                    bmask[:, half * 4 + j, :] = 1.0
        in_maps.append({
            "xtf": xtf_c,
            "xto": xto_c,
            "wqt": wqt,
            "wkt": wkt,
            "wvt": wvt,
            "bmask": bmask.astype(bf),
        })
    return in_maps


def kernel(**inputs):
    x = np.asarray(inputs["inputs"], np.float32)
    Wq = np.asarray(inputs["Wq"], np.float32)
    Wk = np.asarray(inputs["Wk"], np.float32)
    Wv = np.asarray(inputs["Wv"], np.float32)

    if "nc" not in _built:
        _built["nc"] = _build_nc()
    nc = _built["nc"]

    in_maps = _host_inputs(x, Wq, Wk, Wv)
    res = run_bass_kernel_spmd(nc, in_maps, core_ids=list(range(8)))

    out = np.empty((B, S, D), np.float32)
    for c in range(8):
        b, p = c // 2, c % 2
        yc = res.results[c]["y"].astype(np.float32).reshape(NLB, P, D)
        ob = out[b].reshape(NB, P, D)
        for j in range(NLB):
            ob[2 * j + p] = yc[j]
    return out


# revision 10
# speedup vs baseline: 1.1645x; 1.0040x over previous
"""Causal attention kernel for 8 TRN2 NeuronCores.

Problem: B=4, S=4096, D=1024 single-head causal attention with QKV projection.
  q/k/v = x @ W{q,k,v}.T ; out = softmax(tril(q k^T)/sqrt(D)) @ v

Sharding: core c -> batch b = c//2, parity p = c%2. Each core owns the 16 seq
blocks (128 rows) of batch b with block-index parity p ("striped" sequence
parallelism -> balanced causal work). Q and V are projected fused in a single
pass over the core's own rows; v halves are exchanged between the two cores of
a batch with pair-wise AllGathers issued mid-pass (hidden under the remaining
projection matmuls). No K projection: scores come from s^T = x^T . H with
H = (q Wk)^T built per attention group.

v2 changes over the first working version:
  - Q and V passes fused over one x-chunk stream (x loaded once, not twice);
    the two v AllGathers are issued at 50%/100% of the projection pass so both
    complete long before the PV matmuls need them.
  - Head staging: the first matmul only waits for wq's ec=0 slice (256 KB) and
    per-dc x pieces instead of a 5 MB serialized preload.
  - Causal band trimming: band key blocks only compute score columns q >= j'*128
    (variable-width matmuls) and PV only accumulates the covered q blocks. The
    per-core causal pattern is pushed into data (a [128, 8, 128] sub-block mask:
    triangle on the diagonal, ones/zeros off-diagonal depending on parity) so
    the SPMD program stays identical on all cores.
  - Output written bf16 (host upcasts) halving the tail DMA.
  - Per-(qb,eh) PV eviction as soon as that accumulator's last key block is
    done, overlapping the output DMA with the remaining PV matmuls.
"""

import sys
import types

import numpy as np

sys.path.insert(0, "/opt/trn_rl_repo")

try:
    import antenv.axon_hooks  # noqa: F401
except ImportError:
    _hook_mod = types.ModuleType("antenv.axon_hooks")
    _hook_mod._hook = None
    _hook_mod.set_axon_ntff_profile_hook = (
        lambda h: setattr(_hook_mod, "_hook", h)
    )
    _hook_mod.get_axon_ntff_profile_hook = lambda: _hook_mod._hook
    sys.modules["antenv.axon_hooks"] = _hook_mod

import concourse.bass as bass  # noqa: E402
import concourse.mybir as mybir  # noqa: E402
import concourse.tile as tile  # noqa: E402
from concourse import bacc  # noqa: E402
from concourse.bass_utils import run_bass_kernel_spmd  # noqa: E402
from concourse.masks import make_identity  # noqa: E402

import ml_dtypes  # noqa: E402

B, S, D = 4, 4096, 1024
P = 128
NB = S // P          # 32 seq blocks per batch
NLB = NB // 2        # 16 own blocks per core
SH = S // 2          # 2048 own rows per core
NG = 4               # attention q-groups of 512 rows (4 local blocks each)
SCALE = 1.0 / 32.0   # 1/sqrt(D)

BF16 = mybir.dt.bfloat16
F32 = mybir.dt.float32

_built = {}


def _kb_schedule(g):
    """Key-block schedule for group g: list of (half, o, q0, mr).
    q0: first valid score column (columns < q0 are skipped).
    mr: sub-block mask row (half*4+j') applied to cols [q0, q0+128), or None.
    Ordered q0-ascending so the first block covers every q column."""
    kbs = []
    for o in range(4 * g):            # full blocks, no mask
        for half in (0, 1):
            kbs.append((half, o, 0, None))
    for j in range(4):                # band blocks
        for half in (0, 1):
            kbs.append((half, 4 * g + j, j * P, half * 4 + j))
    return kbs


def _build_nc():
    nc = bacc.Bacc("TRN2", target_bir_lowering=False, debug=False, num_devices=8)

    # All large inputs are laid out partition-major by the host so that each
    # DMA is 128 contiguous per-partition descriptors.
    xtf = nc.declare_dram_parameter("xtf", [8, P, 8 * 512], BF16, isOutput=False)
    xto = nc.declare_dram_parameter("xto", [4, P, 8 * 512], BF16, isOutput=False)
    wqt = nc.declare_dram_parameter("wqt", [P, 8, 8, P], BF16, isOutput=False)
    wkt = nc.declare_dram_parameter("wkt", [P, 8, D], BF16, isOutput=False)
    wvt = nc.declare_dram_parameter("wvt", [P, 8, D], BF16, isOutput=False)
    bmask = nc.declare_dram_parameter("bmask", [P, 8, P], BF16, isOutput=False)
    y = nc.declare_dram_parameter("y", [SH, D], BF16, isOutput=True)

    xtf3 = xtf.ap().rearrange("c p (po s) -> c p po s", po=8)   # [8, 128, 8, 512]
    xto3 = xto.ap().rearrange("c p (po s) -> c p po s", po=8)   # [4, 128, 8, 512]
    wqt3 = wqt.ap()
    wkt3 = wkt.ap()
    wvt3 = wvt.ap()
    bmask3 = bmask.ap()
    y3 = y.ap().rearrange("(nb pi) e -> nb pi e", pi=P)         # [16, 128, 1024]

    PAIRS = [[0, 1], [2, 3], [4, 5], [6, 7]]

    with tile.TileContext(nc) as tc:
        with (
            tc.tile_pool(name="dram", bufs=1, space="DRAM") as dram,
            tc.tile_pool(name="consts", bufs=1) as consts,
            tc.tile_pool(name="wqp", bufs=1) as wqp,
            tc.tile_pool(name="wkp", bufs=1) as wkp,
            tc.tile_pool(name="wvp", bufs=1) as wvp,
            tc.tile_pool(name="hp", bufs=1) as hp,
            tc.tile_pool(name="xtp", bufs=2) as xtp,
            tc.tile_pool(name="qgp", bufs=2) as qgp,
            tc.tile_pool(name="ktp", bufs=1) as ktp,
            tc.tile_pool(name="stg", bufs=3) as stg,
            tc.tile_pool(name="strip", bufs=32) as strip,
            tc.tile_pool(name="vload", bufs=4) as vload,
            tc.tile_pool(name="linvp", bufs=2) as linvp,
            tc.tile_pool(name="ctxs", bufs=3) as ctxs,
            tc.tile_pool(name="psum", bufs=8, space="PSUM") as psum,
        ):
            v_own = dram.tile([NLB, P, D], BF16, tag="v_own", name="v_own")
            v_all_a = dram.tile([NLB, P, D], BF16, tag="v_all_a", name="v_all_a")
            v_all_b = dram.tile([NLB, P, D], BF16, tag="v_all_b", name="v_all_b")
            qt_dram = dram.tile([NG, P, 8, 512], BF16, tag="qt_dram", name="qt_dram")

            tri_sb = consts.tile([P, 8, P], BF16)
            ones_sb = consts.tile([P, P], BF16)
            nc.gpsimd.memset(ones_sb[:], 1.0)
            ident_sb = consts.tile([P, P], F32)
            make_identity(nc, ident_sb[:])

            xt_sb = ktp.tile([P, 8, S], BF16)        # x^T: [d, all 4096 rows]

            # ---- Head staging. Each DMA ring entry costs ~0.5 us of ring
            # latency on top of its transfer time, so the head is cut into
            # just enough pieces that the PE's chunk-0 chase (ec groups every
            # 1.7 us, then the V dc-chase) is never more than one piece ahead
            # of the ring: wq ec=0 first, x chunk 0 in 4 pieces, the other wq
            # ec slices individually, then wv in eh halves. wk is NOT in the
            # head -- it is first read ~120 us in by the H matmuls and is
            # inserted into the ring between the chunk-1 and chunk-2 eviction
            # writes instead.
            wq_sb = wqp.tile([P, 8, 8, P], BF16, name="wq_sb")
            nc.sync.dma_start(wq_sb[:, 0], wqt3[:, 0])
            xt_first = xtp.tile([P, 8, 512], BF16, tag="xt", name="xt_first")
            nc.sync.dma_start(xt_first[:], xto3[0])
            nc.sync.dma_start(wq_sb[:, 1:8], wqt3[:, 1:8])
            wv_sb = wvp.tile([P, 8, D], BF16, name="wv_sb")
            nc.sync.dma_start(wv_sb[:, :, 0:512], wvt3[:, :, 0:512])
            nc.sync.dma_start(wv_sb[:, :, 512:D], wvt3[:, :, 512:D])
            wk_sb = wkp.tile([P, 8, D], BF16, name="wk_sb")

            # band masks on the otherwise-idle scalar ring. The full-batch
            # x^T pieces all go on the fast sync ring, staged between the
            # per-chunk eviction writes, keeping the gpsimd ring EMPTY so the
            # two gather triggers fire the moment their v_own halves land.
            nc.scalar.dma_start(tri_sb[:], bmask3)

            # ---- Fused Q+V projection pass over the core's own rows.
            qg_tiles = {}

            def load_qg(g):
                qg = qgp.tile([P, 8, 512], BF16, tag="qg", name=f"qg_{g}")
                nc.scalar.dma_start(qg[:], qt_dram[g])
                qg_tiles[g] = qg

            xt_t = xt_first
            for c in range(4):
                if c < 3:  # prefetch next chunk (sync ring, one entry)
                    xt_n = xtp.tile([P, 8, 512], BF16, tag="xt", name="xt_t")
                    nc.sync.dma_start(xt_n[:], xto3[c + 1])
                # Q: out q^T[e, s] for this 512-row chunk
                for ec in range(8):
                    ps = psum.tile([P, 512], F32, tag="bank", name="ps_q")
                    for dc in range(8):
                        nc.tensor.matmul(
                            ps[:],
                            lhsT=wq_sb[:, ec, dc],
                            rhs=xt_t[:, dc, :],
                            start=(dc == 0),
                            stop=(dc == 7),
                        )
                    qs = stg.tile([P, 512], BF16, tag="stg512", name="qs")
                    nc.scalar.mul(qs[:], ps[:], 1.0)
                    nc.sync.dma_start(qt_dram[c, :, ec, :], qs[:])
                # V: out v[s, e] for the same chunk
                for eh in range(2):
                    for sb in range(4):
                        ps = psum.tile([P, 512], F32, tag="bank", name="ps_v")
                        for dc in range(8):
                            nc.tensor.matmul(
                                ps[:],
                                lhsT=xt_t[:, dc, sb * P:(sb + 1) * P],
                                rhs=wv_sb[:, dc, eh * 512:(eh + 1) * 512],
                                start=(dc == 0),
                                stop=(dc == 7),
                            )
                        vho = stg.tile([P, 512], BF16, tag="stg512", name="vho")
                        nc.vector.tensor_copy(out=vho[:], in_=ps[:])
                        nc.sync.dma_start(
                            v_own[c * 4 + sb][:, eh * 512:(eh + 1) * 512], vho[:]
                        )
                if c == 0:
                    load_qg(0)
                for cc in (c, c + 4):  # two x^T pieces per chunk iteration
                    nc.sync.dma_start(
                        xt_sb[:, :, cc * 512:(cc + 1) * 512], xtf3[cc])
                if c == 1:
                    # first half-gather (own blocks 0-7) issued mid-pass so it
                    # completes well before PV group 0 reads it
                    nc.gpsimd.collective_compute(
                        "AllGather",
                        mybir.AluOpType.bypass,
                        replica_groups=PAIRS,
                        ins=[v_own[0:8].opt()],
                        outs=[v_all_a[:].opt()],
                    )
                if c == 1:
                    nc.sync.dma_start(wk_sb[:], wkt3)
                if c == 2:
                    load_qg(1)
                if c == 3:
                    nc.gpsimd.collective_compute(
                        "AllGather",
                        mybir.AluOpType.bypass,
                        replica_groups=PAIRS,
                        ins=[v_own[8:16].opt()],
                        outs=[v_all_b[:].opt()],
                    )
                xt_t = xt_n

            # ---- Attention ----
            def pass1(g):
                """QK + exp + mask + denominator for group g."""
                kbs = _kb_schedule(g)
                nkb = len(kbs)
                qg = qg_tiles[g]

                # H[d, qi] = sum_e Wk[e, d] q[qi, e], evicted bf16 to SBUF
                h_sb = hp.tile([P, 8, 512], BF16, tag="h", name=f"h_{g}")
                for db in range(8):
                    hps = psum.tile([P, 512], F32, tag="bank", name=f"hps_{g}_{db}")
                    for ec in range(8):
                        nc.tensor.matmul(
                            hps[:],
                            lhsT=wk_sb[:, ec, db * P:(db + 1) * P],
                            rhs=qg[:, ec, :],
                            start=(ec == 0),
                            stop=(ec == 7),
                        )
                    nc.vector.tensor_copy(out=h_sb[:, db, :], in_=hps[:])

                if g + 2 < NG:
                    load_qg(g + 2)

                lrep_ps = psum.tile([P, 512], F32, tag="bank", name=f"lrep_{g}")
                pts = []

                def l_accum(kb_idx):
                    # denominator: column sums replicated across partitions,
                    # issued one key block late so the PE never waits on the
                    # exp/mask of the block it just produced. Column ranges
                    # vary per block; kb 0 is full-width with start=True so
                    # every column's has_written bit is set before any
                    # narrower accumulation lands on it.
                    q0 = kbs[kb_idx][2]
                    nc.tensor.matmul(
                        lrep_ps[:, q0:],
                        lhsT=ones_sb[:],
                        rhs=pts[kb_idx][:, q0:],
                        start=(kb_idx == 0),
                        stop=(kb_idx == nkb - 1),
                        skip_group_check=True,
                    )

                for kb_idx, (half, o, q0, mr) in enumerate(kbs):
                    kcol = half * SH + o * P
                    st_ps = psum.tile([P, 512], F32, tag="bank", name=f"st_ps_{g}")
                    for dc in range(8):
                        nc.tensor.matmul(
                            st_ps[:, q0:],
                            lhsT=xt_sb[:, dc, kcol:kcol + P],
                            rhs=h_sb[:, dc, q0:],
                            start=(dc == 0),
                            stop=(dc == 7),
                        )
                    pt = strip.tile([P, 512], BF16, tag="pt", name=f"pt_{g}")
                    nc.scalar.activation(
                        pt[:, q0:], st_ps[:, q0:],
                        mybir.ActivationFunctionType.Exp, scale=SCALE
                    )
                    if mr is not None:  # mask cols [q0, q0+128): tri/ones/zeros
                        nc.vector.tensor_mul(
                            out=pt[:, q0:q0 + P], in0=pt[:, q0:q0 + P],
                            in1=tri_sb[:, mr, :],
                        )
                    pts.append(pt)
                    if kb_idx >= 1:
                        l_accum(kb_idx - 1)
                l_accum(nkb - 1)

                # denominator -> per-partition scalars via PE transpose of the
                # row-replicated sums; 1/l folds into the ctx eviction scale.
                lsb = linvp.tile([P, 512], F32, tag="lsb", bufs=1, name=f"lsb_{g}")
                nc.vector.tensor_copy(out=lsb[:], in_=lrep_ps[:])
                linv_col = []
                for qb in range(4):
                    ltr = psum.tile([P, P], F32, tag="bank", name=f"ltr_{g}_{qb}")
                    nc.tensor.transpose(ltr[:], lsb[:, qb * P:(qb + 1) * P], ident_sb[:])
                    lc = linvp.tile([P, 1], F32, tag="linv", bufs=8, name=f"linv_{g}_{qb}")
                    nc.vector.reciprocal(lc[:], ltr[:, 0:1])
                    linv_col.append(lc)
                return kbs, pts, linv_col

            def pv(g, state):
                kbs, pts, linv_col = state
                # last key block covering each q sub-block (kb covers qb iff
                # its q0 <= qb*128); evict that accumulator right after.
                last_cover = [max(i for i, kb in enumerate(kbs) if kb[2] <= qb * P)
                              for qb in range(4)]
                ctx_ps = {
                    (qb, eh): psum.tile([P, 512], F32, tag="bank",
                                        name=f"ctx_{g}_{qb}_{eh}")
                    for qb in range(4) for eh in range(2)
                }
                ectr = 0
                for kb_idx, (half, o, q0, mr) in enumerate(kbs):
                    vsrc = v_all_a if o < 8 else v_all_b
                    vb = half * 8 + (o % 8)
                    vt = vload.tile([P, D], BF16, tag="vt", name=f"vt_{g}")
                    # gpsimd: these DMAs wait on the v AllGather semaphore; on
                    # the in-order sync DMA stream they would head-of-line
                    # block later DMAs.
                    nc.gpsimd.dma_start(vt[:], vsrc[vb])
                    for qb in range(4):
                        if q0 > qb * P:
                            continue  # this key block covers no rows of qb
                        for eh in range(2):
                            nc.tensor.matmul(
                                ctx_ps[(qb, eh)][:],
                                lhsT=pts[kb_idx][:, qb * P:(qb + 1) * P],
                                rhs=vt[:, eh * 512:(eh + 1) * 512],
                                start=(kb_idx == 0),
                                stop=(kb_idx == last_cover[qb]),
                            )
                    for qb in range(4):
                        if last_cover[qb] != kb_idx:
                            continue
                        for eh in range(2):
                            cs = ctxs.tile([P, 512], BF16, tag="cs", name=f"cs_{g}")
                            # normalize during eviction; alternate engines so
                            # PSUM banks free ~2x faster
                            if ectr % 2 == 0:
                                nc.scalar.mul(cs[:], ctx_ps[(qb, eh)][:], linv_col[qb][:])
                            else:
                                nc.vector.tensor_scalar_mul(
                                    cs[:], ctx_ps[(qb, eh)][:], linv_col[qb][:])
                            ectr += 1
                            nc.sync.dma_start(
                                y3[4 * g + qb, :, eh * 512:(eh + 1) * 512], cs[:]
                            )

            for g in range(NG):
                pv(g, pass1(g))

    nc.compile()
    return nc


def _host_inputs(x, Wq, Wk, Wv):
    """Build per-core input maps. x: [B,S,D] f32; W*: [D,D] f32."""
    bf = ml_dtypes.bfloat16
    # wq: [pi, ec, dc, e''] with element = Wq[ec*128+e'', dc*128+pi]
    wqt = np.ascontiguousarray(
        Wq.T.astype(bf).reshape(8, P, 8, P).transpose(1, 2, 0, 3)
    )
    # wk: [pi, ec, d] with element = Wk[ec*128+pi, d]
    wkt = np.ascontiguousarray(Wk.astype(bf).reshape(8, P, D).transpose(1, 0, 2))
    # wv: [pi, dc, e] with element = Wv^T[dc*128+pi, e] = Wv[e, dc*128+pi]
    wvt = np.ascontiguousarray(Wv.T.astype(bf).reshape(8, P, D).transpose(1, 0, 2))

    kj = np.arange(P)[:, None]
    qrow = np.arange(P)[None, :]
    tri = (kj <= qrow).astype(np.float32)

    in_maps = []
    xb_cache = {}
    for c in range(8):
        b, p = c // 2, c % 2
        if b not in xb_cache:
            # parity order: [even blocks | odd blocks]
            perm = [2 * j for j in range(NLB)] + [2 * j + 1 for j in range(NLB)]
            xbf = x[b].reshape(NB, P, D)[perm].reshape(S, D)
            xb_cache[b] = xbf.T.astype(bf)  # [D, S]
        xt_full = xb_cache[b]
        # [c, pi, po*512]: per-partition-contiguous chunks
        xtf_c = np.ascontiguousarray(
            xt_full.reshape(8, P, 8, 512).transpose(2, 1, 0, 3)
        ).reshape(8, P, 8 * 512)
        xto_half = xt_full[:, p * SH:(p + 1) * SH]
        xto_c = np.ascontiguousarray(
            xto_half.reshape(8, P, 4, 512).transpose(2, 1, 0, 3)
        ).reshape(4, P, 8 * 512)

        # band sub-block masks [128 kj, 8 r=half*4+j', 128 qrow]:
        # half==p: diagonal triangle; half<p: keep all; half>p: drop all.
        bmask = np.zeros((P, 8, P), np.float32)
        for half in (0, 1):
            for j in range(4):
                if half == p:
                    bmask[:, half * 4 + j, :] = tri
                elif half < p:
                    bmask[:, half * 4 + j, :] = 1.0
        in_maps.append({
            "xtf": xtf_c,
            "xto": xto_c,
            "wqt": wqt,
            "wkt": wkt,
            "wvt": wvt,
            "bmask": bmask.astype(bf),
        })
    return in_maps


def kernel(**inputs):
    x = np.asarray(inputs["inputs"], np.float32)
    Wq = np.asarray(inputs["Wq"], np.float32)
    Wk = np.asarray(inputs["Wk"], np.float32)
    Wv = np.asarray(inputs["Wv"], np.float32)

    if "nc" not in _built:
        _built["nc"] = _build_nc()
    nc = _built["nc"]

    in_maps = _host_inputs(x, Wq, Wk, Wv)
    res = run_bass_kernel_spmd(nc, in_maps, core_ids=list(range(8)))

    out = np.empty((B, S, D), np.float32)
    for c in range(8):
        b, p = c // 2, c % 2
        yc = res.results[c]["y"].astype(np.float32).reshape(NLB, P, D)
        ob = out[b].reshape(NB, P, D)
        for j in range(NLB):
            ob[2 * j + p] = yc[j]
    return out
